# revision 5
# baseline (speedup 1.0000x reference)
"""Beamformer (MoE-style per-frame beam dispatch) for Trainium2, 8 NeuronCores.

Math per frame n (w = W[beam_id[n]]):
    out_r[n,f] = sum_c xr*wr + xi*wi
    out_i[n,f] = sum_c xi*wr - xr*wi          -> out (16384, 2, 257, 1) fp32

Strategy (bf16 wire format; gate is rel_err < 2e-2, bf16 lands ~4e-3):
  * Frames globally sorted by beam on the host -> per-beam contiguous spans;
    the per-frame weight gather becomes static per-beam segments. Bins are
    sharded: core c owns bins [32c, 32c+32) as 4 bingroups of 8 bins; bin 256
    is done on host (1/257 of the work, keeps the SPMD program identical).
  * All DMA payloads are bf16 with one contiguous run per partition per
    transfer (descriptor-efficient): x is packed per chunk as [128, GPC*fch],
    the block-diagonal weight bank ships prebuilt as [128, NTW, 32].
  * Matmul: contraction K=128 (8 bins x re/im x 8 ch), stationary [128, 32]
    per (beam, bingroup). Bingroup pairs share one 32-col PE tile position
    (even bingroup -> stationary cols 0-15, odd -> 16-31, zeros elsewhere)
    accumulated via start/stop into the same PSUM rows -> 64 PSUM partitions
    all real. PSUM->SBUF copies (cast to bf16) alternate DVE/ACT.
  * Uneven chunks (3904x3, 3200, 1024, 448 frames): big early chunks give
    ~31KB DMA descriptors (one per partition per chunk, near wire speed);
    the small tail shrinks the pipeline drain after the last x transfer.
    Output staged [128, sum(fch)/2] (half-chunks stacked on the partition
    axis), written per 2-chunk group so out DMAs overlap compute. Deep
    pools (x bufs=4, staging bufs=3) keep 4 transfers queued on the DGE
    ring, absorbing HBM-contention jitter from the other 7 cores.
"""

import numpy as np

NUM_BEAM, NUM_BIN, NUM_CH = 24, 257, 8
N_FRAMES = 16384
NCORES = 8
P = 128
NBIN_DEV = 256                # bins computed on device
NGRP = NBIN_DEV // 8          # 32 bingroups of 8 bins
GPC = NGRP // NCORES          # 4 bingroups per core
NTW = NUM_BEAM * GPC          # 96 weight tiles per core
CH_SIZES = [3904, 3904, 3904, 3200, 1024, 448]  # frames per chunk (sum = 16384)
NCH = len(CH_SIZES)
F0 = np.concatenate([[0], np.cumsum(CH_SIZES)]).astype(int)   # frame offsets
OC = (F0 // 2).astype(int)    # output column offsets (half-chunk width)
GROUPS = [(0, 2), (2, 4), (4, 6)]     # chunk ranges per out staging/DMA
MAXN = 512                    # max matmul moving dim (one PSUM bank, fp32)

_CACHE = {}
TRACE = False
LAST_RESULTS = None


def _segments(offs):
    """Per (chunk, half): list of (beam, lo, hi) pieces (local cols, <=MAXN)."""
    out = []
    for q in range(NCH):
        hf = CH_SIZES[q] // 2
        for half in range(2):
            n0 = F0[q] + half * hf
            n1 = n0 + hf
            segs = []
            for b in range(NUM_BEAM):
                s0, s1 = max(offs[b], n0), min(offs[b + 1], n1)
                if s1 <= s0:
                    continue
                L = s1 - s0
                npieces = -(-L // MAXN)
                bounds = [s0 + (L * i) // npieces for i in range(npieces + 1)]
                for i in range(npieces):
                    segs.append((b, bounds[i] - n0, bounds[i + 1] - n0))
            out.append(segs)
    return out


def _build_program(offs):
    import concourse.bacc as bacc
    import concourse.bass as bass
    import concourse.tile as tile
    from concourse import mybir

    f32 = mybir.dt.float32
    bf16 = mybir.dt.bfloat16
    halves = _segments(offs)

    nc = bacc.Bacc("TRN2", target_bir_lowering=False, debug=False)
    xt_d = nc.dram_tensor("xt", [P, GPC * N_FRAMES], bf16, kind="ExternalInput")
    wt_d = nc.dram_tensor("wt", [P, NTW, 32], bf16, kind="ExternalInput")
    out_d = nc.dram_tensor("out", [P, N_FRAMES // 2], bf16, kind="ExternalOutput")

    with tile.TileContext(nc) as tc:
        with (
            tc.tile_pool(name="singles", bufs=1) as singles,
            tc.tile_pool(name="xp", bufs=4) as xp,
            tc.tile_pool(name="st", bufs=3) as stp,
            tc.tile_pool(name="ps", bufs=8, space=bass.MemorySpace.PSUM) as ps,
        ):
            w_bank = singles.tile([P, NTW, 32], bf16)
            nc.scalar.dma_start(out=w_bank[:], in_=wt_d[:])

            g_of = {}
            for (g0, g1) in GROUPS:
                for q in range(g0, g1):
                    g_of[q] = (g0, g1)

            ncopy = 0
            st = None
            for q in range(NCH):
                fch = CH_SIZES[q]
                hf = fch // 2
                x_sb = xp.tile([P, GPC, fch], bf16, tag="x")
                nc.sync.dma_start(
                    out=x_sb[:],
                    in_=xt_d[:, GPC * F0[q] : GPC * F0[q + 1]],
                )
                g0, g1 = g_of[q]
                if q == g0:
                    st = stp.tile([P, OC[g1] - OC[g0]], bf16, tag="st")
                for half in range(2):
                    for b, lo, hi in halves[2 * q + half]:
                        pl = hi - lo
                        acc = ps.tile([P, MAXN], f32, tag="acc")
                        for j in range(GPC):
                            h = j // 2
                            nc.tensor.matmul(
                                acc[32 * h : 32 * h + 32, :pl],
                                w_bank[:, b * GPC + j, :],
                                x_sb[:, j, half * hf + lo : half * hf + hi],
                                start=(j % 2 == 0),
                                stop=(j % 2 == 1),
                                tile_position=(0, 32 * h),
                            )
                        co = OC[q] - OC[g0]
                        dst = st[64 * half : 64 * half + 64, co + lo : co + hi]
                        if ncopy % 2 == 0:
                            nc.vector.tensor_copy(dst, acc[:64, :pl])
                        else:
                            nc.scalar.copy(out=dst, in_=acc[:64, :pl])
                        ncopy += 1
                if q == g1 - 1:
                    nc.scalar.dma_start(
                        out=out_d[:, OC[g0] : OC[g1]], in_=st[:]
                    )

    nc.compile()
    return nc


def _pack_weights(W):
    """Per-core block-diagonal stationary banks, each (128, NTW, 32) bf16.

    Row p = fs*16 + (ri*8 + ch); tile tw = b*GPC + j; col = (j%2)*16 + fs*2 + ri'
    holds the conjugate filter-and-sum coefficients:
      ri'=0: [wr | wi],  ri'=1: [-wi | wr]  (stacked over ch in the row dim).
    """
    import ml_dtypes

    wr = W[:, 0]  # (24, 257, 8)
    wi = W[:, 1]
    w16 = np.zeros((NUM_BEAM, NGRP, 8, 16, 2), np.float32)  # b, g, fs, k, ri'
    for g in range(NGRP):
        for fs in range(8):
            fb = g * 8 + fs
            w16[:, g, fs, 0:8, 0] = wr[:, fb]
            w16[:, g, fs, 8:16, 0] = wi[:, fb]
            w16[:, g, fs, 0:8, 1] = -wi[:, fb]
            w16[:, g, fs, 8:16, 1] = wr[:, fb]
    out = []
    for c in range(NCORES):
        bank = np.zeros((P, NTW, 32), np.float32)
        for j in range(GPC):
            g = c * GPC + j
            coff = (j % 2) * 16
            for fs in range(8):
                bank[fs * 16 : (fs + 1) * 16, j::GPC, coff + fs * 2 : coff + fs * 2 + 2] = (
                    w16[:, g, fs].transpose(1, 0, 2)
                )
        out.append(bank.astype(ml_dtypes.bfloat16))
    return out


def _pack_x(inp, perm):
    """Per-core x tensors [128, GPC*N_FRAMES] bf16, per-chunk contiguous:
    partition p = fs*16 + ri*8 + ch; chunk q occupies cols
    [GPC*F0[q], GPC*F0[q+1]) as [g, frame-in-chunk]."""
    import ml_dtypes

    xb = np.asarray(inp, dtype=np.float32).astype(ml_dtypes.bfloat16)
    xs = xb[perm][:, :, :NBIN_DEV, :]  # (N, 2, 256, 8) bf16
    # (n, ri, c, g, fs, ch) -> [c, fs, ri, ch, g, n]
    arr = xs.reshape(N_FRAMES, 2, NCORES, GPC, 8, NUM_CH)
    arr = np.ascontiguousarray(arr.transpose(2, 4, 1, 5, 3, 0))
    arr = arr.reshape(NCORES, P, GPC, N_FRAMES)
    out = np.empty((NCORES, P, GPC * N_FRAMES), ml_dtypes.bfloat16)
    for q in range(NCH):
        blk = arr[:, :, :, F0[q] : F0[q + 1]]  # (NCORES, P, GPC, fch)
        out[:, :, GPC * F0[q] : GPC * F0[q + 1]] = blk.reshape(
            NCORES, P, GPC * CH_SIZES[q]
        )
    return out


def kernel(**inputs):
    global LAST_RESULTS
    from concourse.bass_utils import run_bass_kernel_spmd

    inp = np.ascontiguousarray(np.asarray(inputs["input"], dtype=np.float32))
    W = np.ascontiguousarray(np.asarray(inputs["W"], dtype=np.float32))
    bid = np.asarray(inputs["beam_id"]).astype(np.int64)

    perm = np.argsort(bid, kind="stable")
    counts = np.bincount(bid, minlength=NUM_BEAM)
    offs = np.concatenate([[0], np.cumsum(counts)]).astype(int)

    key = tuple(offs)
    if key not in _CACHE:
        _CACHE[key] = _build_program(offs)
    nc = _CACHE[key]

    wts = _pack_weights(W)
    xts = _pack_x(inp, perm)
    in_maps = [{"xt": xts[c], "wt": wts[c]} for c in range(NCORES)]

    res = run_bass_kernel_spmd(nc, in_maps, list(range(NCORES)), trace=TRACE)
    LAST_RESULTS = res

    # staging row (within half-chunk) = 64*half + 16*j + fs*2 + ri
    out_sorted = np.empty((N_FRAMES, 2, NUM_BIN), np.float32)
    for c in range(NCORES):
        ot = np.asarray(res.results[c]["out"]).astype(np.float32)  # (128, N/2)
        for q in range(NCH):
            hf = CH_SIZES[q] // 2
            blk = ot[:, OC[q] : OC[q + 1]]           # (128, hf)
            a = blk.reshape(2, GPC, 8, 2, hf)        # (half, j, fs, ri, f2)
            a = a.transpose(0, 4, 3, 1, 2)           # (half, f2, ri, j, fs)
            out_sorted[F0[q] : F0[q + 1], :, 32 * c : 32 * c + 32] = a.reshape(
                CH_SIZES[q], 2, 32
            )

    # bin 256 on host (keeps the device bin count divisible by 8 cores)
    xs = inp[:, :, NUM_BIN - 1, :]
    ws = W[bid][:, :, NUM_BIN - 1, :]
    xr, xi = xs[:, 0], xs[:, 1]
    wr, wi = ws[:, 0], ws[:, 1]

    out_full = np.empty((N_FRAMES, 2, NUM_BIN), np.float32)
    out_full[perm] = out_sorted
    out_full[:, 0, NUM_BIN - 1] = (xr * wr + xi * wi).sum(-1)
    out_full[:, 1, NUM_BIN - 1] = (xi * wr - xr * wi).sum(-1)
    return out_full.reshape(N_FRAMES, 2, NUM_BIN, 1)
